# revision 18
# baseline (speedup 1.0000x reference)
"""CLSTMCell fused cell kernel for 8 Trainium2 NeuronCores.

Data-parallel over the batch: each of the 8 cores processes a 512-row batch
shard; weights are replicated.

The pre-activations have complex-multiplication structure. With
a = [x_r h_r], b = [x_i h_i]  (each [512, 2048]) and stacked weights
Wr = [R; Rr], Wi = [I; Ir]  (each [2048, 4096]):
    zr = a @ Wr + b @ Wi + br
    zi = b @ Wr - a @ Wi + bi
Karatsuba 3-product form (25% less tensor work than the 4-product form):
    m1 = a @ Wr            (fp32r)
    q  = b @ Wi            (bf16 - the only reduced-precision product)
    m3 = (a+b) @ (Wr-Wi)   (fp32r)
    zr = m1 + q,  zi = m3 - m1 + q
Per gate g (i,f,c,o): i,f,o -> hard_sigmoid, c~ -> tanh, then
    c = f*c_prev + i*tanh(c~);  h = o*tanh(c)
(The first U output columns use zr's gates, the last U use zi's.)

Device layout: output columns on PSUM partitions, batch on the free dim.
Work is organized in 32 groups (8 column-phases x 4 gates); each group
accumulates three 16-step psum chains (m1/q/m3) from [128k,128n] stationary
weight tiles and [128k,512b] moving activation blocks, then a short
DVE/ACT combine drains the three banks into the gate activation. s = a+b
is computed on device from the quartered a/b tiles. All DMA descriptors
are >=2KB per partition line; weights stream per-group (2MB fp32 + 0.5MB
bf16), double-prefetched two groups ahead.
"""

import sys

sys.path.insert(0, "/opt/trn_rl_repo")

import ml_dtypes
import numpy as np

import concourse.bacc as bacc
import concourse.mybir as mybir
import concourse.tile as tile
from concourse.bass_utils import run_bass_kernel_spmd

N_CORES = 8
B, D, U = 4096, 1024, 1024
BS = B // N_CORES          # batch rows per core
P = 128                    # SBUF partitions
KB = (D + U) // P          # 16 contraction blocks of 128
NT = U // P                # 8 column-phases per gate
NGRP = NT * 4              # 32 (phase, gate) groups
QK = 2                     # act tiles span 2 k-blocks each
F32 = mybir.dt.float32
F32R = mybir.dt.float32r
BF16 = mybir.dt.bfloat16
ADD = mybir.AluOpType.add
SUB = mybir.AluOpType.subtract
MULT = mybir.AluOpType.mult
MIN = mybir.AluOpType.min
NPBF16 = ml_dtypes.bfloat16

_CACHE = {}


def _build():
    nc = bacc.Bacc("TRN2", target_bir_lowering=False, debug=False,
                   num_devices=N_CORES)
    Tanh = mybir.ActivationFunctionType.Tanh
    Relu = mybir.ActivationFunctionType.Relu

    din = {}
    din["aT"] = nc.dram_tensor("aT", [P, KB * BS], F32R,
                               kind="ExternalInput").ap()
    din["bT"] = nc.dram_tensor("bT", [P, KB * BS], BF16,
                               kind="ExternalInput").ap()
    din["wf"] = nc.dram_tensor("wf", [NGRP * P, KB * 2 * P], F32R,
                               kind="ExternalInput").ap()
    din["wq"] = nc.dram_tensor("wq", [NGRP * P, KB * P], BF16,
                               kind="ExternalInput").ap()
    din["c_prevT"] = nc.dram_tensor("c_prevT", [2 * U, BS], F32,
                                    kind="ExternalInput").ap()
    din["brT"] = nc.dram_tensor("brT", [P, NGRP // 1], F32,
                                kind="ExternalInput").ap()
    din["biT"] = nc.dram_tensor("biT", [P, NGRP // 1], F32,
                                kind="ExternalInput").ap()
    h_outT = nc.dram_tensor("h_outT", [2 * U, BS], F32,
                            kind="ExternalOutput").ap()
    c_outT = nc.dram_tensor("c_outT", [2 * U, BS], F32,
                            kind="ExternalOutput").ap()

    with tile.TileContext(nc) as tc:
        with (
            tc.tile_pool(name="apool", bufs=KB // QK) as apool,
            tc.tile_pool(name="bpool", bufs=KB // QK) as bpool,
            tc.tile_pool(name="spool", bufs=KB // QK) as spool,
            tc.tile_pool(name="wfp", bufs=6) as wfp,
            tc.tile_pool(name="wqp", bufs=3) as wqp,
            tc.tile_pool(name="bias", bufs=4) as bias_p,
            tc.tile_pool(name="gatep", bufs=2) as gate_p,
            tc.tile_pool(name="cprev", bufs=4) as cpool,
            tc.tile_pool(name="comb", bufs=7) as comb_p,
            tc.tile_pool(name="gat", bufs=11) as gat_p,
            tc.tile_pool(name="tmp", bufs=6) as tmp_p,
            tc.tile_pool(name="outs", bufs=4) as out_p,
            tc.tile_pool(name="psum", bufs=8, space="PSUM") as psum_p,
        ):
            QW = QK * BS  # columns per act chunk-tile

            # --- weight prefetch: 2 fp32 tiles + 1 bf16 tile per group ----
            wtiles = {}

            def fetch_w(gi, eng, parts="qab"):
                r0 = gi * P
                half = KB * P
                if "q" in parts:
                    wq_t = wqp.tile([P, KB * P], BF16, tag="wq",
                                    name=f"wq{gi}")
                    eng.dma_start(wq_t[:], din["wq"][r0:r0 + P, :])
                    wtiles[(gi, "q")] = wq_t
                if "a" in parts:
                    wfa = wfp.tile([P, KB * P], F32R, tag="wf",
                                   name=f"wfa{gi}")
                    eng.dma_start(wfa[:], din["wf"][r0:r0 + P, :half])
                    wtiles[(gi, "a")] = wfa
                if "b" in parts:
                    wfb = wfp.tile([P, KB * P], F32R, tag="wf",
                                   name=f"wfb{gi}")
                    eng.dma_start(wfb[:], din["wf"][r0:r0 + P, half:])
                    wtiles[(gi, "b")] = wfb

            # --- resident activation chunks; s = a + b on device ----------
            # DMA priority order matches first-use order: group 0's q
            # operands (wq + b) first, then its m1/m3 weights interleaved
            # with the a chunks, then groups 1-2.
            a_q, b_q, s_q = [], [], []

            def dma_b(j):
                bt = bpool.tile([P, QW], BF16, tag="b", name=f"b{j}")
                nc.sync.dma_start(bt[:], din["bT"][:, j * QW:(j + 1) * QW])
                b_q.append(bt)

            def dma_a(j):
                at = apool.tile([P, QW], F32R, tag="a", name=f"a{j}")
                nc.sync.dma_start(at[:], din["aT"][:, j * QW:(j + 1) * QW])
                a_q.append(at)

            # sync queue streams group 0's data in exact first-use order
            # with exclusive bandwidth; groups 1-2 follow on the ACT/Pool
            # queues, gated by tiny reads of early act chunks so they don't
            # steal bandwidth from group 0's critical prefix.
            fetch_w(0, nc.sync, "q")
            for j in range(8):
                dma_b(j)
            fetch_w(0, nc.sync, "a")
            for j in range(4):
                dma_a(j)
            fetch_w(0, nc.sync, "b")
            for j in range(4, 8):
                dma_a(j)
            gate1 = gate_p.tile([P, 1], F32, tag="gate", name="gate1")
            nc.scalar.activation(gate1[:], b_q[1][:, 0:1],
                                 mybir.ActivationFunctionType.Copy)
            fetch_w(1, nc.scalar)
            gate2 = gate_p.tile([P, 1], F32, tag="gate", name="gate2")
            nc.gpsimd.tensor_copy(gate2[:], a_q[1][:, 0:1])
            fetch_w(2, nc.gpsimd)

            def amov(k):
                return a_q[k // QK][:, (k % QK) * BS:(k % QK + 1) * BS]

            def bmov(k):
                return b_q[k // QK][:, (k % QK) * BS:(k % QK + 1) * BS]

            def smov(k):
                return s_q[k // QK][:, (k % QK) * BS:(k % QK + 1) * BS]

            for j in range(KB // QK):
                st = spool.tile([P, QW], F32R, tag="s", name=f"s{j}")
                nc.vector.tensor_tensor(st[:], a_q[j][:], b_q[j][:], ADD)
                s_q.append(st)

            # --- per-partition bias tiles [128, 32]; col = g*8 + t --------
            braw, bhs = [], []

            def emit_bias():
                for name in ("brT", "biT"):
                    t = bias_p.tile([P, NGRP], F32, tag="bias",
                                    name=f"braw_{name}")
                    nc.scalar.dma_start(t[:], din[name][:, :])
                    braw.append(t)
                    t2 = bias_p.tile([P, NGRP], F32, tag="bias",
                                     name=f"bhs_{name}")
                    nc.vector.tensor_scalar(t2[:], t[:], 0.2, 0.5, MULT, ADD)
                    bhs.append(t2)

            # --- main loop: 8 phases x 4 gates -----------------------------
            for t in range(NT):
                cps = {}
                for z in range(2):
                    cp = cpool.tile([P, BS], F32, tag="cprev",
                                    name=f"cp_{t}_{z}")
                    rows0 = z * U + t * P
                    nc.gpsimd.dma_start(cp[:],
                                        din["c_prevT"][rows0:rows0 + P, :])
                    cps[z] = cp
                gacts = {}
                tc2s = {}
                for g in range(4):
                    gi = t * 4 + g
                    if 2 < gi + 2 < NGRP:
                        fetch_w(gi + 2, nc.sync)
                    wfa = wtiles.pop((gi, "a"))
                    wfb = wtiles.pop((gi, "b"))
                    wq_t = wtiles.pop((gi, "q"))
                    m1 = psum_p.tile([P, BS], F32, tag="ps",
                                     name=f"m1_{gi}")
                    qp = psum_p.tile([P, BS], F32, tag="ps", name=f"q_{gi}")
                    m3 = psum_p.tile([P, BS], F32, tag="ps",
                                     name=f"m3_{gi}")

                    # one dtype-sequential chain per psum bank; even groups
                    # run q|m1|m3, odd groups m1|m3|q so consecutive groups
                    # keep the PE in the same precision mode at the seam
                    def chain_q():
                        for k in range(KB):
                            nc.tensor.matmul(
                                qp[:], wq_t[:, k * P:(k + 1) * P], bmov(k),
                                start=(k == 0), stop=(k == KB - 1))

                    def chain_m(ps, mov, off):
                        for k in range(KB):
                            wt = wfa if k < 8 else wfb
                            c0 = 2 * (k % 8) * P + off
                            nc.tensor.matmul(
                                ps[:], wt[:, c0:c0 + P], mov(k),
                                start=(k == 0), stop=(k == KB - 1))

                    if gi % 2 == 0 or gi == NGRP - 1:
                        # last group also runs q first so only the short
                        # m3+combine tail trails the final matmul
                        chain_q()
                        chain_m(m1, amov, 0)
                        chain_m(m3, smov, P)
                    else:
                        chain_m(m1, amov, 0)
                        chain_m(m3, smov, P)
                        chain_q()
                    if not braw:
                        emit_bias()
                    # drain the three banks: zr = m1+q, zi = m3+q-m1
                    qs = comb_p.tile([P, BS], F32, tag="comb",
                                     name=f"qs_{gi}")
                    nc.vector.tensor_copy(qs[:], qp[:])
                    zr = comb_p.tile([P, BS], F32, tag="comb",
                                     name=f"zr_{gi}")
                    nc.vector.tensor_tensor(zr[:], m1[:], qs[:], ADD)
                    t0 = comb_p.tile([P, BS], F32, tag="comb",
                                     name=f"t0_{gi}")
                    nc.vector.tensor_tensor(t0[:], m3[:], qs[:], ADD)
                    zi = comb_p.tile([P, BS], F32, tag="comb",
                                     name=f"zi_{gi}")
                    nc.vector.tensor_tensor(zi[:], t0[:], m1[:], SUB)
                    col = g * NT + t
                    for z, zz in enumerate((zr, zi)):
                        gt = gat_p.tile([P, BS], F32, tag="gat",
                                        name=f"g_{gi}_{z}")
                        if g == 2:
                            nc.scalar.activation(
                                gt[:], zz[:], Tanh,
                                bias=braw[z][:, col:col + 1], scale=1.0)
                        else:
                            # relu(0.2*z + 0.2*b + 0.5); min(.,1) rides the
                            # consuming DVE op
                            nc.scalar.activation(
                                gt[:], zz[:], Relu,
                                bias=bhs[z][:, col:col + 1], scale=0.2)
                        gacts[(g, z)] = gt
                    if g == 2:
                        for z in range(2):
                            rows0 = z * U + t * P
                            t1 = tmp_p.tile([P, BS], F32, tag="tmp",
                                            name=f"t1_{t}_{z}")
                            nc.vector.scalar_tensor_tensor(
                                t1[:], gacts[(1, z)][:], 1.0, cps[z][:],
                                MIN, MULT)
                            t2 = tmp_p.tile([P, BS], F32, tag="tmp",
                                            name=f"t2_{t}_{z}")
                            nc.vector.scalar_tensor_tensor(
                                t2[:], gacts[(0, z)][:], 1.0,
                                gacts[(2, z)][:], MIN, MULT)
                            cn = out_p.tile([P, BS], F32, tag="out",
                                            name=f"cn_{t}_{z}")
                            nc.vector.tensor_tensor(cn[:], t1[:], t2[:], ADD)
                            nc.gpsimd.dma_start(
                                c_outT[rows0:rows0 + P, :], cn[:])
                            tc2 = tmp_p.tile([P, BS], F32, tag="tmp",
                                             name=f"tc2_{t}_{z}")
                            nc.scalar.activation(tc2[:], cn[:], Tanh)
                            tc2s[z] = tc2
                    if g == 3:
                        for z in range(2):
                            rows0 = z * U + t * P
                            if t == NT - 1 and z == 1:
                                # kernel tail: half-batch chunks pipeline
                                # the final mul + DMA
                                hn = out_p.tile([P, BS], F32, tag="out",
                                                name=f"hn_{t}_{z}")
                                for h0 in (0, BS // 2):
                                    sl = slice(h0, h0 + BS // 2)
                                    nc.vector.scalar_tensor_tensor(
                                        hn[:, sl], gacts[(3, z)][:, sl],
                                        1.0, tc2s[z][:, sl], MIN, MULT)
                                    nc.gpsimd.dma_start(
                                        h_outT[rows0:rows0 + P, sl],
                                        hn[:, sl])
                                continue
                            hn = out_p.tile([P, BS], F32, tag="out",
                                            name=f"hn_{t}_{z}")
                            nc.vector.scalar_tensor_tensor(
                                hn[:], gacts[(3, z)][:], 1.0, tc2s[z][:],
                                MIN, MULT)
                            nc.gpsimd.dma_start(
                                h_outT[rows0:rows0 + P, :], hn[:])

    nc.compile()
    return nc


def _in_maps(inputs, h_tm1, c_tm1, wr, wi, wrr, wir, br, bi):
    Wr = np.vstack([wr, wrr])            # [2048, 4096]
    Wi = np.vstack([wi, wir])
    Wd = Wr - Wi

    def perm(W):  # [2048, 4096] -> [t, g, p, k, c]
        return W.reshape(KB, P, 4, NT, P).transpose(3, 2, 1, 0, 4)

    wf = np.stack([perm(Wr), perm(Wd)], axis=4)      # [t,g,p,k,2,c]
    wf = np.ascontiguousarray(
        wf.reshape(NGRP * P, KB * 2 * P), dtype=np.float32)
    wq = np.ascontiguousarray(
        perm(Wi).reshape(NGRP * P, KB * P)).astype(NPBF16)
    brT = np.ascontiguousarray(
        br.reshape(4, NT, P).transpose(2, 0, 1).reshape(P, NGRP))
    biT = np.ascontiguousarray(
        bi.reshape(4, NT, P).transpose(2, 0, 1).reshape(P, NGRP))

    def actperm(m, dt):  # [512, 2048] -> [128, 16*512], part = k-part
        v = m.T.reshape(KB, P, BS).transpose(1, 0, 2).reshape(P, KB * BS)
        return np.ascontiguousarray(v).astype(dt)

    maps = []
    for c in range(N_CORES):
        rows = slice(c * BS, (c + 1) * BS)
        a = np.hstack([inputs[rows, :D], h_tm1[rows, :U]])
        b = np.hstack([inputs[rows, D:], h_tm1[rows, U:]])
        maps.append({
            "aT": actperm(a, np.float32),
            "bT": actperm(b, NPBF16),
            "c_prevT": np.ascontiguousarray(c_tm1[rows].T),
            "wf": wf, "wq": wq,
            "brT": brT, "biT": biT,
        })
    return maps


def kernel(inputs, h_tm1, c_tm1, real_kernel, imaginary_kernel,
           real_recurrent_kernel, imaginary_recurrent_kernel,
           real_bias, imaginary_bias):
    if "nc" not in _CACHE:
        _CACHE["nc"] = _build()
    nc = _CACHE["nc"]

    maps = _in_maps(
        np.ascontiguousarray(inputs, dtype=np.float32),
        np.ascontiguousarray(h_tm1, dtype=np.float32),
        np.ascontiguousarray(c_tm1, dtype=np.float32),
        np.ascontiguousarray(real_kernel, dtype=np.float32),
        np.ascontiguousarray(imaginary_kernel, dtype=np.float32),
        np.ascontiguousarray(real_recurrent_kernel, dtype=np.float32),
        np.ascontiguousarray(imaginary_recurrent_kernel, dtype=np.float32),
        np.ascontiguousarray(real_bias, dtype=np.float32),
        np.ascontiguousarray(imaginary_bias, dtype=np.float32),
    )
    res = run_bass_kernel_spmd(nc, maps, list(range(N_CORES)))
    h = np.concatenate(
        [res.results[c]["h_outT"].T for c in range(N_CORES)], axis=0)
    c = np.concatenate(
        [res.results[c]["c_outT"].T for c in range(N_CORES)], axis=0)
    return np.ascontiguousarray(h), np.ascontiguousarray(c)


# revision 24
# speedup vs baseline: 1.0362x; 1.0362x over previous
"""CLSTMCell fused cell kernel for 8 Trainium2 NeuronCores.

Data-parallel over the batch: each of the 8 cores processes a 512-row batch
shard; weights are replicated.

The pre-activations have complex-multiplication structure. With
a = [x_r h_r], b = [x_i h_i]  (each [512, 2048]) and stacked weights
Wr = [R; Rr], Wi = [I; Ir]  (each [2048, 4096]):
    zr = a @ Wr + b @ Wi + br
    zi = b @ Wr - a @ Wi + bi
Karatsuba 3-product form (25% less tensor work than the 4-product form):
    m1 = a @ Wr            (fp32r)
    q  = b @ Wi            (bf16 - the only reduced-precision product)
    m3 = (a+b) @ (Wr-Wi)   (fp32r)
    zr = m1 + q,  zi = m3 - m1 + q
Per gate g (i,f,c,o): i,f,o -> hard_sigmoid, c~ -> tanh, then
    c = f*c_prev + i*tanh(c~);  h = o*tanh(c)
(The first U output columns use zr's gates, the last U use zi's.)

Device layout: output columns on PSUM partitions, batch on the free dim.
Work is organized in 32 groups (8 column-phases x 4 gates); each group
accumulates three 16-step psum chains (m1/q/m3) from [128k,128n] stationary
weight tiles and [128k,512b] moving activation blocks, then a short
DVE/ACT combine drains the three banks into the gate activation. s = a+b
is computed on device from the quartered a/b tiles. All DMA descriptors
are >=2KB per partition line; weights stream per-group (2MB fp32 + 0.5MB
bf16), double-prefetched two groups ahead.
"""

import sys

sys.path.insert(0, "/opt/trn_rl_repo")

import ml_dtypes
import numpy as np

import concourse.bacc as bacc
import concourse.mybir as mybir
import concourse.tile as tile
from concourse.bass_utils import run_bass_kernel_spmd

N_CORES = 8
B, D, U = 4096, 1024, 1024
BS = B // N_CORES          # batch rows per core
P = 128                    # SBUF partitions
KB = (D + U) // P          # 16 contraction blocks of 128
NT = U // P                # 8 column-phases per gate
NGRP = NT * 4              # 32 (phase, gate) groups
QK = 2                     # act tiles span 2 k-blocks each
F32 = mybir.dt.float32
F32R = mybir.dt.float32r
BF16 = mybir.dt.bfloat16
ADD = mybir.AluOpType.add
SUB = mybir.AluOpType.subtract
MULT = mybir.AluOpType.mult
MIN = mybir.AluOpType.min
NPBF16 = ml_dtypes.bfloat16

_CACHE = {}


def _build():
    nc = bacc.Bacc("TRN2", target_bir_lowering=False, debug=False,
                   num_devices=N_CORES)
    Tanh = mybir.ActivationFunctionType.Tanh
    Relu = mybir.ActivationFunctionType.Relu

    din = {}
    din["aT"] = nc.dram_tensor("aT", [P, KB * BS], F32R,
                               kind="ExternalInput").ap()
    din["bT"] = nc.dram_tensor("bT", [P, KB * BS], BF16,
                               kind="ExternalInput").ap()
    din["wf"] = nc.dram_tensor("wf", [NGRP * P, KB * 2 * P], F32R,
                               kind="ExternalInput").ap()
    din["wq"] = nc.dram_tensor("wq", [NGRP * P, KB * P], BF16,
                               kind="ExternalInput").ap()
    din["c_prevT"] = nc.dram_tensor("c_prevT", [2 * U, BS], F32,
                                    kind="ExternalInput").ap()
    din["brT"] = nc.dram_tensor("brT", [P, NGRP // 1], F32,
                                kind="ExternalInput").ap()
    din["biT"] = nc.dram_tensor("biT", [P, NGRP // 1], F32,
                                kind="ExternalInput").ap()
    h_outT = nc.dram_tensor("h_outT", [2 * U, BS], F32,
                            kind="ExternalOutput").ap()
    c_outT = nc.dram_tensor("c_outT", [2 * U, BS], F32,
                            kind="ExternalOutput").ap()

    with tile.TileContext(nc) as tc:
        with (
            tc.tile_pool(name="apool", bufs=KB // QK) as apool,
            tc.tile_pool(name="bpool", bufs=KB // QK) as bpool,
            tc.tile_pool(name="spool", bufs=KB // QK) as spool,
            tc.tile_pool(name="wfp", bufs=6) as wfp,
            tc.tile_pool(name="wqp", bufs=6) as wqp,
            tc.tile_pool(name="bias", bufs=4) as bias_p,
            tc.tile_pool(name="cprev", bufs=4) as cpool,
            tc.tile_pool(name="comb", bufs=7) as comb_p,
            tc.tile_pool(name="gat", bufs=11) as gat_p,
            tc.tile_pool(name="tmp", bufs=6) as tmp_p,
            tc.tile_pool(name="outs", bufs=4) as out_p,
            tc.tile_pool(name="psum", bufs=8, space="PSUM") as psum_p,
        ):
            QW = QK * BS  # columns per act chunk-tile

            # --- weight prefetch: 2 fp32 tiles + 1 bf16 tile per group ----
            wtiles = {}

            def fetch_w(gi, eng, parts="qab"):
                r0 = gi * P
                half = KB * P
                qh = KB * P // 2
                if "q" in parts:
                    wqa = wqp.tile([P, qh], BF16, tag="wq",
                                   name=f"wqa{gi}")
                    wqb = wqp.tile([P, qh], BF16, tag="wq",
                                   name=f"wqb{gi}")
                    eng.dma_start(wqa[:], din["wq"][r0:r0 + P, :qh])
                    eng.dma_start(wqb[:], din["wq"][r0:r0 + P, qh:])
                    wtiles[(gi, "q")] = (wqa, wqb)
                if "a" in parts:
                    wfa = wfp.tile([P, KB * P], F32R, tag="wf",
                                   name=f"wfa{gi}")
                    eng.dma_start(wfa[:], din["wf"][r0:r0 + P, :half])
                    wtiles[(gi, "a")] = wfa
                if "b" in parts:
                    wfb = wfp.tile([P, KB * P], F32R, tag="wf",
                                   name=f"wfb{gi}")
                    eng.dma_start(wfb[:], din["wf"][r0:r0 + P, half:])
                    wtiles[(gi, "b")] = wfb

            # --- resident activation chunks; s = a + b on device ----------
            # DMA priority order matches first-use order: group 0's q
            # operands (wq + b) first, then its m1/m3 weights interleaved
            # with the a chunks, then groups 1-2.
            a_q, b_q, s_q = [], [], []

            def dma_b(j):
                bt = bpool.tile([P, QW], BF16, tag="b", name=f"b{j}")
                nc.sync.dma_start(bt[:], din["bT"][:, j * QW:(j + 1) * QW])
                b_q.append(bt)

            def dma_a(j):
                at = apool.tile([P, QW], F32R, tag="a", name=f"a{j}")
                nc.sync.dma_start(at[:], din["aT"][:, j * QW:(j + 1) * QW])
                a_q.append(at)

            # acts stream on the sync queue; group 0/1 weights race
            # concurrently on the ACT engine's queue
            for j in range(8):
                dma_b(j)
            for j in range(8):
                dma_a(j)
            fetch_w(0, nc.scalar)
            fetch_w(1, nc.scalar)

            def amov(k):
                return a_q[k // QK][:, (k % QK) * BS:(k % QK + 1) * BS]

            def bmov(k):
                return b_q[k // QK][:, (k % QK) * BS:(k % QK + 1) * BS]

            def smov(k):
                return s_q[k // QK][:, (k % QK) * BS:(k % QK + 1) * BS]

            for j in range(KB // QK):
                st = spool.tile([P, QW], F32R, tag="s", name=f"s{j}")
                nc.vector.tensor_tensor(st[:], a_q[j][:], b_q[j][:], ADD)
                s_q.append(st)

            # --- per-partition bias tiles [128, 32]; col = g*8 + t --------
            braw, bhs = [], []

            def emit_bias():
                for name in ("brT", "biT"):
                    t = bias_p.tile([P, NGRP], F32, tag="bias",
                                    name=f"braw_{name}")
                    nc.scalar.dma_start(t[:], din[name][:, :])
                    braw.append(t)
                    t2 = bias_p.tile([P, NGRP], F32, tag="bias",
                                     name=f"bhs_{name}")
                    nc.vector.tensor_scalar(t2[:], t[:], 0.2, 0.5, MULT, ADD)
                    bhs.append(t2)

            # --- main loop: 8 phases x 4 gates -----------------------------
            for t in range(NT):
                cps = {}
                for z in range(2):
                    cp = cpool.tile([P, BS], F32, tag="cprev",
                                    name=f"cp_{t}_{z}")
                    rows0 = z * U + t * P
                    nc.gpsimd.dma_start(cp[:],
                                        din["c_prevT"][rows0:rows0 + P, :])
                    cps[z] = cp
                gacts = {}
                tc2s = {}
                for g in range(4):
                    gi = t * 4 + g
                    if gi + 2 < NGRP:
                        fetch_w(gi + 2, nc.sync)
                    wfa = wtiles.pop((gi, "a"))
                    wfb = wtiles.pop((gi, "b"))
                    wqa, wqb = wtiles.pop((gi, "q"))
                    m1 = psum_p.tile([P, BS], F32, tag="ps",
                                     name=f"m1_{gi}")
                    qp = psum_p.tile([P, BS], F32, tag="ps", name=f"q_{gi}")
                    m3 = psum_p.tile([P, BS], F32, tag="ps",
                                     name=f"m3_{gi}")

                    # one dtype-sequential chain per psum bank; even groups
                    # run q|m1|m3, odd groups m1|m3|q so consecutive groups
                    # keep the PE in the same precision mode at the seam
                    def chain_q():
                        for k in range(KB):
                            wt = wqa if k < 8 else wqb
                            c0 = (k % 8) * P
                            nc.tensor.matmul(
                                qp[:], wt[:, c0:c0 + P], bmov(k),
                                start=(k == 0), stop=(k == KB - 1))

                    def chain_m(ps, mov, off):
                        for k in range(KB):
                            wt = wfa if k < 8 else wfb
                            c0 = 2 * (k % 8) * P + off
                            nc.tensor.matmul(
                                ps[:], wt[:, c0:c0 + P], mov(k),
                                start=(k == 0), stop=(k == KB - 1))

                    if gi % 2 == 0 or gi == NGRP - 1:
                        # last group also runs q first so only the short
                        # m3+combine tail trails the final matmul
                        chain_q()
                        chain_m(m1, amov, 0)
                        chain_m(m3, smov, P)
                    else:
                        chain_m(m1, amov, 0)
                        chain_m(m3, smov, P)
                        chain_q()
                    if not braw:
                        emit_bias()
                    # drain the three banks: zr = m1+q, zi = m3+q-m1
                    qs = comb_p.tile([P, BS], F32, tag="comb",
                                     name=f"qs_{gi}")
                    nc.vector.tensor_copy(qs[:], qp[:])
                    zr = comb_p.tile([P, BS], F32, tag="comb",
                                     name=f"zr_{gi}")
                    nc.vector.tensor_tensor(zr[:], m1[:], qs[:], ADD)
                    t0 = comb_p.tile([P, BS], F32, tag="comb",
                                     name=f"t0_{gi}")
                    nc.vector.tensor_tensor(t0[:], m3[:], qs[:], ADD)
                    zi = comb_p.tile([P, BS], F32, tag="comb",
                                     name=f"zi_{gi}")
                    nc.vector.tensor_tensor(zi[:], t0[:], m1[:], SUB)
                    col = g * NT + t
                    for z, zz in enumerate((zr, zi)):
                        gt = gat_p.tile([P, BS], F32, tag="gat",
                                        name=f"g_{gi}_{z}")
                        if g == 2:
                            nc.scalar.activation(
                                gt[:], zz[:], Tanh,
                                bias=braw[z][:, col:col + 1], scale=1.0)
                        else:
                            # relu(0.2*z + 0.2*b + 0.5); min(.,1) rides the
                            # consuming DVE op
                            nc.scalar.activation(
                                gt[:], zz[:], Relu,
                                bias=bhs[z][:, col:col + 1], scale=0.2)
                        gacts[(g, z)] = gt
                    if g == 2:
                        for z in range(2):
                            rows0 = z * U + t * P
                            t1 = tmp_p.tile([P, BS], F32, tag="tmp",
                                            name=f"t1_{t}_{z}")
                            nc.vector.scalar_tensor_tensor(
                                t1[:], gacts[(1, z)][:], 1.0, cps[z][:],
                                MIN, MULT)
                            t2 = tmp_p.tile([P, BS], F32, tag="tmp",
                                            name=f"t2_{t}_{z}")
                            nc.vector.scalar_tensor_tensor(
                                t2[:], gacts[(0, z)][:], 1.0,
                                gacts[(2, z)][:], MIN, MULT)
                            cn = out_p.tile([P, BS], F32, tag="out",
                                            name=f"cn_{t}_{z}")
                            nc.vector.tensor_tensor(cn[:], t1[:], t2[:], ADD)
                            nc.gpsimd.dma_start(
                                c_outT[rows0:rows0 + P, :], cn[:])
                            tc2 = tmp_p.tile([P, BS], F32, tag="tmp",
                                             name=f"tc2_{t}_{z}")
                            nc.scalar.activation(tc2[:], cn[:], Tanh)
                            tc2s[z] = tc2
                    if g == 3:
                        for z in range(2):
                            rows0 = z * U + t * P
                            if t == NT - 1 and z == 1:
                                # kernel tail: half-batch chunks pipeline
                                # the final mul + DMA
                                hn = out_p.tile([P, BS], F32, tag="out",
                                                name=f"hn_{t}_{z}")
                                for h0 in (0, BS // 2):
                                    sl = slice(h0, h0 + BS // 2)
                                    nc.vector.scalar_tensor_tensor(
                                        hn[:, sl], gacts[(3, z)][:, sl],
                                        1.0, tc2s[z][:, sl], MIN, MULT)
                                    nc.gpsimd.dma_start(
                                        h_outT[rows0:rows0 + P, sl],
                                        hn[:, sl])
                                continue
                            hn = out_p.tile([P, BS], F32, tag="out",
                                            name=f"hn_{t}_{z}")
                            nc.vector.scalar_tensor_tensor(
                                hn[:], gacts[(3, z)][:], 1.0, tc2s[z][:],
                                MIN, MULT)
                            nc.gpsimd.dma_start(
                                h_outT[rows0:rows0 + P, :], hn[:])

    nc.compile()
    return nc


def _in_maps(inputs, h_tm1, c_tm1, wr, wi, wrr, wir, br, bi):
    Wr = np.vstack([wr, wrr])            # [2048, 4096]
    Wi = np.vstack([wi, wir])
    Wd = Wr - Wi

    def perm(W):  # [2048, 4096] -> [t, g, p, k, c]
        return W.reshape(KB, P, 4, NT, P).transpose(3, 2, 1, 0, 4)

    wf = np.stack([perm(Wr), perm(Wd)], axis=4)      # [t,g,p,k,2,c]
    wf = np.ascontiguousarray(
        wf.reshape(NGRP * P, KB * 2 * P), dtype=np.float32)
    wq = np.ascontiguousarray(
        perm(Wi).reshape(NGRP * P, KB * P)).astype(NPBF16)
    brT = np.ascontiguousarray(
        br.reshape(4, NT, P).transpose(2, 0, 1).reshape(P, NGRP))
    biT = np.ascontiguousarray(
        bi.reshape(4, NT, P).transpose(2, 0, 1).reshape(P, NGRP))

    def actperm(m, dt):  # [512, 2048] -> [128, 16*512], part = k-part
        v = m.T.reshape(KB, P, BS).transpose(1, 0, 2).reshape(P, KB * BS)
        return np.ascontiguousarray(v).astype(dt)

    maps = []
    for c in range(N_CORES):
        rows = slice(c * BS, (c + 1) * BS)
        a = np.hstack([inputs[rows, :D], h_tm1[rows, :U]])
        b = np.hstack([inputs[rows, D:], h_tm1[rows, U:]])
        maps.append({
            "aT": actperm(a, np.float32),
            "bT": actperm(b, NPBF16),
            "c_prevT": np.ascontiguousarray(c_tm1[rows].T),
            "wf": wf, "wq": wq,
            "brT": brT, "biT": biT,
        })
    return maps


def kernel(inputs, h_tm1, c_tm1, real_kernel, imaginary_kernel,
           real_recurrent_kernel, imaginary_recurrent_kernel,
           real_bias, imaginary_bias):
    if "nc" not in _CACHE:
        _CACHE["nc"] = _build()
    nc = _CACHE["nc"]

    maps = _in_maps(
        np.ascontiguousarray(inputs, dtype=np.float32),
        np.ascontiguousarray(h_tm1, dtype=np.float32),
        np.ascontiguousarray(c_tm1, dtype=np.float32),
        np.ascontiguousarray(real_kernel, dtype=np.float32),
        np.ascontiguousarray(imaginary_kernel, dtype=np.float32),
        np.ascontiguousarray(real_recurrent_kernel, dtype=np.float32),
        np.ascontiguousarray(imaginary_recurrent_kernel, dtype=np.float32),
        np.ascontiguousarray(real_bias, dtype=np.float32),
        np.ascontiguousarray(imaginary_bias, dtype=np.float32),
    )
    res = run_bass_kernel_spmd(nc, maps, list(range(N_CORES)))
    h = np.concatenate(
        [res.results[c]["h_outT"].T for c in range(N_CORES)], axis=0)
    c = np.concatenate(
        [res.results[c]["c_outT"].T for c in range(N_CORES)], axis=0)
    return np.ascontiguousarray(h), np.ascontiguousarray(c)


# revision 25
# speedup vs baseline: 1.0864x; 1.0484x over previous
"""CLSTMCell fused cell kernel for 8 Trainium2 NeuronCores.

Data-parallel over the batch: each of the 8 cores processes a 512-row batch
shard; weights are replicated.

The pre-activations have complex-multiplication structure. With
a = [x_r h_r], b = [x_i h_i]  (each [512, 2048]) and stacked weights
Wr = [R; Rr], Wi = [I; Ir]  (each [2048, 4096]):
    zr = a @ Wr + b @ Wi + br
    zi = b @ Wr - a @ Wi + bi
Karatsuba 3-product form (25% less tensor work than the 4-product form):
    q  = b @ Wi
    m1 = a @ Wr
    m3 = (a+b) @ (Wr-Wi)
    zr = m1 + q,  zi = m3 - m1 + q
All matmul operands are fp16 (10-bit mantissa; h max rel err ~2e-3 vs the
fp32 reference) with fp32 PSUM accumulation — full-rate on the PE and half
the HBM traffic of fp32. Per gate g (i,f,c,o): i,f,o -> hard_sigmoid,
c~ -> tanh, then c = f*c_prev + i*tanh(c~); h = o*tanh(c). The first U
output columns use zr's gates, the last U use zi's.

Device layout: output columns on PSUM partitions, batch on the free dim.
Work is organized in 32 groups (8 column-phases x 4 gates); each group
accumulates three dtype-uniform 16-step psum chains (q/m1/m3) from
[128k,128n] stationary weight tiles and [128k,512b] moving activation
blocks, then a short DVE/ACT combine drains the three banks into the gate
activations. s = a+b is computed on device from the chunked a/b tiles.
Weights stream per-group as six 256KB DMAs (2KB per partition line),
prefetched three groups ahead; group 0/1 weights race the act stream on
the ACT engine's DGE queue.
"""

import sys

sys.path.insert(0, "/opt/trn_rl_repo")

import numpy as np

import concourse.bacc as bacc
import concourse.mybir as mybir
import concourse.tile as tile
from concourse.bass_utils import run_bass_kernel_spmd

N_CORES = 8
B, D, U = 4096, 1024, 1024
BS = B // N_CORES          # batch rows per core
P = 128                    # SBUF partitions
KB = (D + U) // P          # 16 contraction blocks of 128
NT = U // P                # 8 column-phases per gate
NGRP = NT * 4              # 32 (phase, gate) groups
QK = 2                     # act tiles span 2 k-blocks each
F32 = mybir.dt.float32
F16 = mybir.dt.float16
ADD = mybir.AluOpType.add
SUB = mybir.AluOpType.subtract
MULT = mybir.AluOpType.mult
MIN = mybir.AluOpType.min
WCOLS = 3 * KB * P         # weight dram cols per group row-block

_CACHE = {}


def _build():
    nc = bacc.Bacc("TRN2", target_bir_lowering=False, debug=False,
                   num_devices=N_CORES)
    Tanh = mybir.ActivationFunctionType.Tanh
    Relu = mybir.ActivationFunctionType.Relu

    din = {}
    din["aT"] = nc.dram_tensor("aT", [P, KB * BS], F16,
                               kind="ExternalInput").ap()
    din["bT"] = nc.dram_tensor("bT", [P, KB * BS], F16,
                               kind="ExternalInput").ap()
    din["w16"] = nc.dram_tensor("w16", [NGRP * P, WCOLS], F16,
                                kind="ExternalInput").ap()
    din["c_prevT"] = nc.dram_tensor("c_prevT", [2 * U, BS], F16,
                                    kind="ExternalInput").ap()
    din["brT"] = nc.dram_tensor("brT", [P, NGRP], F32,
                                kind="ExternalInput").ap()
    din["biT"] = nc.dram_tensor("biT", [P, NGRP], F32,
                                kind="ExternalInput").ap()
    h_outT = nc.dram_tensor("h_outT", [2 * U, BS], F32,
                            kind="ExternalOutput").ap()
    c_outT = nc.dram_tensor("c_outT", [2 * U, BS], F32,
                            kind="ExternalOutput").ap()

    with tile.TileContext(nc) as tc:
        with (
            tc.tile_pool(name="apool", bufs=KB // QK) as apool,
            tc.tile_pool(name="bpool", bufs=KB // QK) as bpool,
            tc.tile_pool(name="spool", bufs=KB // QK) as spool,
            tc.tile_pool(name="wp", bufs=24) as wp,
            tc.tile_pool(name="bias", bufs=4) as bias_p,
            tc.tile_pool(name="cprev", bufs=4) as cpool,
            tc.tile_pool(name="comb", bufs=8) as comb_p,
            tc.tile_pool(name="gat", bufs=12) as gat_p,
            tc.tile_pool(name="tmp", bufs=8) as tmp_p,
            tc.tile_pool(name="outs", bufs=6) as out_p,
            tc.tile_pool(name="psum", bufs=8, space="PSUM") as psum_p,
        ):
            QW = QK * BS  # columns per act chunk-tile
            HK = KB * P // 2  # weight cols per half-chain tile

            # --- weight prefetch: 3 chains x 2 half-tiles per group -------
            wtiles = {}

            def fetch_w(gi, eng):
                r0 = gi * P
                for ci, c in enumerate(("q", "a", "s")):
                    base = ci * KB * P
                    lo = wp.tile([P, HK], F16, tag="w", name=f"w{c}l{gi}")
                    hi = wp.tile([P, HK], F16, tag="w", name=f"w{c}h{gi}")
                    eng.dma_start(lo[:],
                                  din["w16"][r0:r0 + P, base:base + HK])
                    eng.dma_start(hi[:],
                                  din["w16"][r0:r0 + P, base + HK:base + 2 * HK])
                    wtiles[(gi, c)] = (lo, hi)

            # --- resident activation chunks; s = a + b on device ----------
            a_q, b_q, s_q = [], [], []

            def dma_b(j):
                bt = bpool.tile([P, QW], F16, tag="b", name=f"b{j}")
                nc.sync.dma_start(bt[:], din["bT"][:, j * QW:(j + 1) * QW])
                b_q.append(bt)

            def dma_a(j):
                at = apool.tile([P, QW], F16, tag="a", name=f"a{j}")
                nc.sync.dma_start(at[:], din["aT"][:, j * QW:(j + 1) * QW])
                a_q.append(at)

            # acts stream on the sync queue; group 0-2 weights race
            # concurrently on the ACT engine's queue
            for j in range(8):
                dma_b(j)
            for j in range(8):
                dma_a(j)
            fetch_w(0, nc.scalar)
            fetch_w(1, nc.scalar)
            fetch_w(2, nc.scalar)

            def amov(k):
                return a_q[k // QK][:, (k % QK) * BS:(k % QK + 1) * BS]

            def bmov(k):
                return b_q[k // QK][:, (k % QK) * BS:(k % QK + 1) * BS]

            def smov(k):
                return s_q[k // QK][:, (k % QK) * BS:(k % QK + 1) * BS]

            for j in range(KB // QK):
                st = spool.tile([P, QW], F16, tag="s", name=f"s{j}")
                nc.vector.tensor_tensor(st[:], a_q[j][:], b_q[j][:], ADD)
                s_q.append(st)

            # --- per-partition bias tiles [128, 32]; col = g*8 + t --------
            braw, bhs = [], []

            def emit_bias():
                for name in ("brT", "biT"):
                    t = bias_p.tile([P, NGRP], F32, tag="bias",
                                    name=f"braw_{name}")
                    nc.scalar.dma_start(t[:], din[name][:, :])
                    braw.append(t)
                    t2 = bias_p.tile([P, NGRP], F32, tag="bias",
                                     name=f"bhs_{name}")
                    nc.vector.tensor_scalar(t2[:], t[:], 0.2, 0.5, MULT, ADD)
                    bhs.append(t2)

            # --- main loop: 8 phases x 4 gates -----------------------------
            for t in range(NT):
                cps = {}
                for z in range(2):
                    cp = cpool.tile([P, BS], F16, tag="cprev",
                                    name=f"cp_{t}_{z}")
                    rows0 = z * U + t * P
                    nc.gpsimd.dma_start(cp[:],
                                        din["c_prevT"][rows0:rows0 + P, :])
                    cps[z] = cp
                gacts = {}
                tc2s = {}
                for g in range(4):
                    gi = t * 4 + g
                    if gi + 3 < NGRP:
                        fetch_w(gi + 3, nc.sync)
                    m1 = psum_p.tile([P, BS], F32, tag="ps",
                                     name=f"m1_{gi}")
                    qp = psum_p.tile([P, BS], F32, tag="ps", name=f"q_{gi}")
                    m3 = psum_p.tile([P, BS], F32, tag="ps",
                                     name=f"m3_{gi}")

                    def chain(ps, wkey, mov, gi=gi):
                        lo, hi = wtiles.pop((gi, wkey))
                        for k in range(KB):
                            wt = lo if k < 8 else hi
                            c0 = (k % 8) * P
                            nc.tensor.matmul(
                                ps[:], wt[:, c0:c0 + P], mov(k),
                                start=(k == 0), stop=(k == KB - 1))

                    chain(qp, "q", bmov)
                    chain(m1, "a", amov)
                    chain(m3, "s", smov)
                    if not braw:
                        emit_bias()
                    # drain the three banks: zr = m1+q, zi = (m3+q)-m1
                    qs = comb_p.tile([P, BS], F32, tag="comb",
                                     name=f"qs_{gi}")
                    nc.vector.tensor_copy(qs[:], qp[:])
                    zr = comb_p.tile([P, BS], F32, tag="comb",
                                     name=f"zr_{gi}")
                    nc.vector.tensor_tensor(zr[:], m1[:], qs[:], ADD)
                    t0 = comb_p.tile([P, BS], F32, tag="comb",
                                     name=f"t0_{gi}")
                    nc.vector.tensor_tensor(t0[:], m3[:], qs[:], ADD)
                    zi = comb_p.tile([P, BS], F32, tag="comb",
                                     name=f"zi_{gi}")
                    nc.vector.tensor_tensor(zi[:], t0[:], m1[:], SUB)
                    col = g * NT + t
                    for z, zz in enumerate((zr, zi)):
                        gt = gat_p.tile([P, BS], F32, tag="gat",
                                        name=f"g_{gi}_{z}")
                        if g == 2:
                            nc.scalar.activation(
                                gt[:], zz[:], Tanh,
                                bias=braw[z][:, col:col + 1], scale=1.0)
                        else:
                            # relu(0.2*z + 0.2*b + 0.5); min(.,1) rides the
                            # consuming DVE op
                            nc.scalar.activation(
                                gt[:], zz[:], Relu,
                                bias=bhs[z][:, col:col + 1], scale=0.2)
                        gacts[(g, z)] = gt
                    if g == 2:
                        for z in range(2):
                            rows0 = z * U + t * P
                            t1 = tmp_p.tile([P, BS], F32, tag="tmp",
                                            name=f"t1_{t}_{z}")
                            nc.vector.scalar_tensor_tensor(
                                t1[:], gacts[(1, z)][:], 1.0, cps[z][:],
                                MIN, MULT)
                            t2 = tmp_p.tile([P, BS], F32, tag="tmp",
                                            name=f"t2_{t}_{z}")
                            nc.vector.scalar_tensor_tensor(
                                t2[:], gacts[(0, z)][:], 1.0,
                                gacts[(2, z)][:], MIN, MULT)
                            cn = out_p.tile([P, BS], F32, tag="out",
                                            name=f"cn_{t}_{z}")
                            nc.vector.tensor_tensor(cn[:], t1[:], t2[:], ADD)
                            nc.gpsimd.dma_start(
                                c_outT[rows0:rows0 + P, :], cn[:])
                            tc2 = tmp_p.tile([P, BS], F32, tag="tmp",
                                             name=f"tc2_{t}_{z}")
                            nc.scalar.activation(tc2[:], cn[:], Tanh)
                            tc2s[z] = tc2
                    if g == 3:
                        for z in range(2):
                            rows0 = z * U + t * P
                            if t == NT - 1 and z == 1:
                                # kernel tail: half-batch chunks pipeline
                                # the final mul + DMA
                                hn = out_p.tile([P, BS], F32, tag="out",
                                                name=f"hn_{t}_{z}")
                                for h0 in (0, BS // 2):
                                    sl = slice(h0, h0 + BS // 2)
                                    nc.vector.scalar_tensor_tensor(
                                        hn[:, sl], gacts[(3, z)][:, sl],
                                        1.0, tc2s[z][:, sl], MIN, MULT)
                                    nc.gpsimd.dma_start(
                                        h_outT[rows0:rows0 + P, sl],
                                        hn[:, sl])
                                continue
                            hn = out_p.tile([P, BS], F32, tag="out",
                                            name=f"hn_{t}_{z}")
                            nc.vector.scalar_tensor_tensor(
                                hn[:], gacts[(3, z)][:], 1.0, tc2s[z][:],
                                MIN, MULT)
                            nc.gpsimd.dma_start(
                                h_outT[rows0:rows0 + P, :], hn[:])

    nc.compile()
    return nc


def _in_maps(inputs, h_tm1, c_tm1, wr, wi, wrr, wir, br, bi):
    Wr = np.vstack([wr, wrr])            # [2048, 4096]
    Wi = np.vstack([wi, wir])
    Wd = Wr - Wi

    def perm(W):  # [2048, 4096] -> [NGRP*P rows, KB*P cols] fp16
        v = W.reshape(KB, P, 4, NT, P).transpose(3, 2, 1, 0, 4)
        return v.reshape(NGRP * P, KB * P)

    w16 = np.ascontiguousarray(
        np.concatenate([perm(Wi), perm(Wr), perm(Wd)], axis=1),
        dtype=np.float16)
    brT = np.ascontiguousarray(
        br.reshape(4, NT, P).transpose(2, 0, 1).reshape(P, NGRP))
    biT = np.ascontiguousarray(
        bi.reshape(4, NT, P).transpose(2, 0, 1).reshape(P, NGRP))

    def actperm(m):  # [512, 2048] -> [128, 16*512] fp16, part = k-part
        v = m.T.reshape(KB, P, BS).transpose(1, 0, 2).reshape(P, KB * BS)
        return np.ascontiguousarray(v, dtype=np.float16)

    maps = []
    for c in range(N_CORES):
        rows = slice(c * BS, (c + 1) * BS)
        a = np.hstack([inputs[rows, :D], h_tm1[rows, :U]])
        b = np.hstack([inputs[rows, D:], h_tm1[rows, U:]])
        maps.append({
            "aT": actperm(a),
            "bT": actperm(b),
            "c_prevT": np.ascontiguousarray(
                c_tm1[rows].T, dtype=np.float16),
            "w16": w16,
            "brT": brT, "biT": biT,
        })
    return maps


def kernel(inputs, h_tm1, c_tm1, real_kernel, imaginary_kernel,
           real_recurrent_kernel, imaginary_recurrent_kernel,
           real_bias, imaginary_bias):
    if "nc" not in _CACHE:
        _CACHE["nc"] = _build()
    nc = _CACHE["nc"]

    maps = _in_maps(
        np.ascontiguousarray(inputs, dtype=np.float32),
        np.ascontiguousarray(h_tm1, dtype=np.float32),
        np.ascontiguousarray(c_tm1, dtype=np.float32),
        np.ascontiguousarray(real_kernel, dtype=np.float32),
        np.ascontiguousarray(imaginary_kernel, dtype=np.float32),
        np.ascontiguousarray(real_recurrent_kernel, dtype=np.float32),
        np.ascontiguousarray(imaginary_recurrent_kernel, dtype=np.float32),
        np.ascontiguousarray(real_bias, dtype=np.float32),
        np.ascontiguousarray(imaginary_bias, dtype=np.float32),
    )
    res = run_bass_kernel_spmd(nc, maps, list(range(N_CORES)))
    h = np.concatenate(
        [res.results[c]["h_outT"].T for c in range(N_CORES)], axis=0)
    c = np.concatenate(
        [res.results[c]["c_outT"].T for c in range(N_CORES)], axis=0)
    return np.ascontiguousarray(h), np.ascontiguousarray(c)
